# revision 20
# baseline (speedup 1.0000x reference)
"""Trainium2 Bass kernel for nn_Cross_modal_Center_ContrastiveLoss.

Math (reference): per-class segment means of two modal feature matrices,
gathered per sample, SmoothL1 against learned centers, mean over [N, D],
summed over the two modalities.

Because every sample of class c contributes the identical per-element loss,
the whole loss reduces to
    loss = (1/(N*D)) * sum_c n_c * sum_d [ f(mean1[c,d]-centers[c,d])
                                         + f(mean2[c,d]-centers[c,d]) ]
so the only O(N*D) work is the segment sums / counts.

Sharding: the host sorts samples by class (a gather, part of input
marshalling), then shards the sorted batch over N across the 8 NeuronCores.
After sorting, each core's 4096-row shard spans only ~50 contiguous classes,
so the per-K-tile one-hot is [128, W] with W = 64 instead of [128, 400] --
the TensorE segment-sum matmuls stream 6x fewer columns. Each core:
  - builds per-K-tile one-hots [128, W] on VectorE (iota == target-base),
  - computes local segment sums via TensorE matmuls accumulated in PSUM
    (out[D-chunk, Wclasses] = Xchunk.T @ onehot, 8 PSUM accumulators),
  - accumulates the one-hots on VectorE and derives counts with one matmul,
  - writes local sums+counts [1025, W] (bf16 sums, exact small-int counts)
    to DRAM.
The host gathers the 8 partial results, scatter-adds them into the global
[classes] axis, and evaluates the tiny [C, D] epilogue. (An on-device
all-reduce was measured at a ~90us fixed floor in this environment -- more
than the entire kernel -- so the cross-core reduction of the small partials
is done at gather time instead.)

bf16 is used for the matmul operands: the one-hot entries (0/1) are exact in
bf16, and rounding the inputs to bf16 perturbs the final loss by ~2e-7
relative (measured) because the per-element rounding errors cancel across
the 16.7M-element reduction. PSUM accumulation is fp32.
"""

import os
import sys

for _p in ("/opt/trn_rl_repo", "/root/.axon_site/_ro/trn_rl_repo"):
    if os.path.isdir(_p) and _p not in sys.path:
        sys.path.append(_p)

import numpy as np

import concourse.tile as tile
from concourse import bass_utils, bacc, mybir

N, D, C = 32768, 512, 395  # batch, feat dim, classes
NCORES = 8
NSH = N // NCORES  # 4096 rows per core
KT = NSH // 128  # 32 K-tiles per core
GROUP_SIZES = [8, 8, 8, 4, 2, 1, 1]  # K-tiles per DMA group: big first for
# max DMA bandwidth, small last so the final arrive-then-consume step is tiny
assert sum(GROUP_SIZES) == KT
OUT_ROWS = 8 * 128 + 1  # 4 D-chunks x 2 modals + counts row

_CACHE = {}


def _build(W):
    fp32 = mybir.dt.float32
    fp16 = mybir.dt.float16
    bf16 = mybir.dt.bfloat16
    nc = bacc.Bacc("TRN2", target_bir_lowering=False, debug=False, num_devices=NCORES)
    # x holds both modals row-interleaved: row r = [modal1[r, :] | modal2[r, :]]
    x = nc.dram_tensor("x", [NSH, 2 * D], bf16, kind="ExternalInput")
    tgt = nc.dram_tensor("tgt", [128, KT], fp32, kind="ExternalInput")
    iota = nc.dram_tensor("iota", [128, W], fp16, kind="ExternalInput")
    out = nc.dram_tensor("out", [OUT_ROWS, W], bf16, kind="ExternalOutput")

    with tile.TileContext(nc) as tc:
        with (
            tc.tile_pool(name="xin", bufs=6) as xin,
            tc.tile_pool(name="single", bufs=1) as single,
            tc.tile_pool(name="psum", bufs=1, space="PSUM") as psum,
        ):
            iota_sb = single.tile([128, W], fp16)
            nc.scalar.dma_start(iota_sb[:], iota.ap())
            tgt_sb = single.tile([128, KT], fp32)
            nc.scalar.dma_start(tgt_sb[:], tgt.ap())
            ones = single.tile([128, 1], bf16)
            nc.vector.memset(ones[:], 1.0)
            oh = single.tile([128, KT, W], bf16)  # all K-tile one-hots
            sums_sb = single.tile([128, 8, W], bf16)
            counts_sb = single.tile([1, W], bf16)

            # acc0's PSUM bank has spare columns (a bank is 512 f32); counts
            # accumulate in cols [W, 2W) of the same bank, row 0.
            accs = [
                psum.tile([128, 2 * W if j == 0 else W], fp32, tag=f"acc{j}", name=f"acc{j}")
                for j in range(8)
            ]

            # partition p holds rows p*KT..p*KT+KT-1 of the shard, so each
            # group DMA is a fully contiguous tpg*2KB run per partition
            xf = x.ap().rearrange("(p r) d -> p r d", p=128)

            k0 = 0
            for g, tpg in enumerate(GROUP_SIZES):
                xg = xin.tile([128, 8, 2 * D], bf16, tag="xg", name="xg")
                nc.sync.dma_start(xg[:, :tpg, :], xf[:, k0 : k0 + tpg, :])
                for t in range(tpg):
                    k = k0 + t
                    ohk = oh[:, k, :]
                    nc.vector.tensor_scalar(
                        ohk,
                        iota_sb[:],
                        tgt_sb[:, k : k + 1],
                        None,
                        mybir.AluOpType.is_equal,
                    )
                    st, sp = (k == 0), (k == KT - 1)
                    for j in range(8):
                        nc.tensor.matmul(
                            accs[j][:, :W] if j == 0 else accs[j][:],
                            lhsT=xg[:, t, j * 128 : (j + 1) * 128],
                            rhs=ohk,
                            start=st,
                            stop=sp,
                        )
                    # counts: ones.T @ onehot into acc0's spare columns.
                    # start=False always: the k==0 modal matmul already cleared
                    # this bank's has_written bits, so the first write lands as
                    # an overwrite and later ones accumulate.
                    nc.tensor.matmul(
                        accs[0][0:1, W : 2 * W],
                        lhsT=ones[:],
                        rhs=ohk,
                        start=False,
                        stop=sp,
                        skip_group_check=True,
                    )
                k0 += tpg

            out_ap = out.ap()
            for j in range(8):
                src_ap = accs[j][:, :W] if j == 0 else accs[j][:]
                if j % 2 == 0:
                    nc.vector.tensor_copy(sums_sb[:, j, :], src_ap)
                else:
                    nc.scalar.copy(sums_sb[:, j, :], src_ap)
            nc.vector.tensor_copy(counts_sb[:], accs[0][0:1, W : 2 * W])
            # one DMA for all 8 sum tiles, partition-major rows (p*8+j) so each
            # partition writes one contiguous 8*W run
            nc.sync.dma_start(
                out_ap[0:1024, :].rearrange("(p j) w -> p j w", p=128), sums_sb[:]
            )
            nc.scalar.dma_start(out_ap[1024:1025, :], counts_sb[:])

    nc.compile()
    return nc


def _get_nc(W):
    key = ("nc", W)
    if key not in _CACHE:
        _CACHE[key] = _build(W)
    return _CACHE[key]


def _make_in_maps(modal1, modal2, targets):
    import ml_dtypes

    tg = np.asarray(targets).astype(np.int64).reshape(N)
    perm = np.argsort(tg, kind="stable")
    tgs = tg[perm]
    xcat = np.empty((N, 2 * D), dtype=ml_dtypes.bfloat16)
    xcat[:, :D] = np.asarray(modal1).astype(ml_dtypes.bfloat16)[perm]
    xcat[:, D:] = np.asarray(modal2).astype(ml_dtypes.bfloat16)[perm]

    bases = [int(tgs[c * NSH]) for c in range(NCORES)]
    maxw = max(int(tgs[(c + 1) * NSH - 1]) - bases[c] + 1 for c in range(NCORES))
    W = max(64, ((maxw + 15) // 16) * 16)

    iota = np.ascontiguousarray(
        np.broadcast_to(np.arange(W, dtype=np.float16), (128, W))
    )
    in_maps = []
    for c in range(NCORES):
        rows = slice(c * NSH, (c + 1) * NSH)
        tgt_c = np.ascontiguousarray(
            (tgs[rows] - bases[c]).reshape(128, KT).astype(np.float32)
        )  # [128, KT]: partition p row t <-> shard row p*KT + t
        in_maps.append(
            {
                "x": np.ascontiguousarray(xcat[rows]),
                "tgt": tgt_c,
                "iota": iota,
            }
        )
    return in_maps, bases, W


def _epilogue(acc, centers):
    # acc: [1025, C+pad] float64 global sums; rows 0..511 modal1 (D-major),
    # rows 512..1023 modal2, row 1024 counts.
    counts = acc[1024, :C]
    clamp = np.maximum(counts, 1.0)
    cT = np.asarray(centers, dtype=np.float64).T  # [D, C]

    def sl1(x):
        d = np.abs(x)
        return np.where(d < 1.0, 0.5 * d * d, d - 0.5)

    total = 0.0
    for base in (0, 512):
        meanT = acc[base : base + 512, :C] / clamp
        total += (sl1(meanT - cT) * counts).sum()
    return np.float32(total / (N * D))


def _run(inputs, trace=False, tmpdir=None):
    in_maps, bases, W = _make_in_maps(
        inputs["modal1_inputs"], inputs["modal2_inputs"], inputs["targets"]
    )
    nc = _get_nc(W)
    kw = {}
    if trace:
        kw = {"trace": True, "tmpdir": tmpdir}
    res = bass_utils.run_bass_kernel_spmd(
        nc, in_maps, core_ids=list(range(NCORES)), **kw
    )
    acc = np.zeros((OUT_ROWS, C + W), dtype=np.float64)
    for c in range(NCORES):
        o = np.asarray(res.results[c]["out"], dtype=np.float64)
        sums = o[0:1024].reshape(128, 8, W).transpose(1, 0, 2).reshape(1024, W)
        acc[0:1024, bases[c] : bases[c] + W] += sums
        acc[1024, bases[c] : bases[c] + W] += o[1024]
    loss = _epilogue(acc, inputs["centers"])
    return loss, res


def kernel(**inputs) -> np.ndarray:
    loss, _ = _run(inputs)
    return loss


def kernel_profiled(**inputs):
    """Like kernel() but returns (loss, BassKernelResults) with NTFF trace."""
    import tempfile
    import types

    # antenv.axon_hooks is missing in this image; shim it so bass_utils can
    # find the NTFF profile hook, and keep artifacts local.
    if "antenv.axon_hooks" not in sys.modules:
        import antenv

        hooks_mod = types.ModuleType("antenv.axon_hooks")
        _h = [None]
        hooks_mod.set_axon_ntff_profile_hook = lambda h: _h.__setitem__(0, h)
        hooks_mod.get_axon_ntff_profile_hook = lambda: _h[0]
        sys.modules["antenv.axon_hooks"] = hooks_mod
        antenv.axon_hooks = hooks_mod
        try:
            from trn_agent_boot.trn_boot import _ntff_profile_via_ctypes

            hooks_mod.set_axon_ntff_profile_hook(
                _ntff_profile_via_ctypes("/opt/axon/libaxon_pjrt.so")
            )
        except Exception as e:
            print(f"profile hook setup failed: {e}", file=sys.stderr)
    bass_utils.upload_artifacts = lambda d: d
    tmpdir = tempfile.mkdtemp(prefix="ccloss_trace_")
    return _run(inputs, trace=True, tmpdir=tmpdir)


# revision 21
# speedup vs baseline: 1.1425x; 1.1425x over previous
"""Trainium2 Bass kernel for nn_Cross_modal_Center_ContrastiveLoss.

Math (reference): per-class segment means of two modal feature matrices,
gathered per sample, SmoothL1 against learned centers, mean over [N, D],
summed over the two modalities.

Because every sample of class c contributes the identical per-element loss,
the whole loss reduces to
    loss = (1/(N*D)) * sum_c n_c * sum_d [ f(mean1[c,d]-centers[c,d])
                                         + f(mean2[c,d]-centers[c,d]) ]
so the only O(N*D) work is the segment sums / counts.

Sharding: the host sorts samples by class (a gather, part of input
marshalling), then shards the sorted batch over N across the 8 NeuronCores.
After sorting, each core's 4096-row shard spans only ~50 contiguous classes,
so the per-K-tile one-hot is [128, W] with W = 64 instead of [128, 400] --
the TensorE segment-sum matmuls stream 6x fewer columns. Each core:
  - builds per-K-tile one-hots [128, W] on VectorE (iota == target-base),
  - computes local segment sums via TensorE matmuls accumulated in PSUM
    (out[D-chunk, Wclasses] = Xchunk.T @ onehot, 8 PSUM accumulators),
  - accumulates the one-hots on VectorE and derives counts with one matmul,
  - writes local sums+counts [1025, W] (bf16 sums, exact small-int counts)
    to DRAM.
The host gathers the 8 partial results, scatter-adds them into the global
[classes] axis, and evaluates the tiny [C, D] epilogue. (An on-device
all-reduce was measured at a ~90us fixed floor in this environment -- more
than the entire kernel -- so the cross-core reduction of the small partials
is done at gather time instead.)

bf16 is used for the matmul operands: the one-hot entries (0/1) are exact in
bf16, and rounding the inputs to bf16 perturbs the final loss by ~2e-7
relative (measured) because the per-element rounding errors cancel across
the 16.7M-element reduction. PSUM accumulation is fp32.
"""

import os
import sys

for _p in ("/opt/trn_rl_repo", "/root/.axon_site/_ro/trn_rl_repo"):
    if os.path.isdir(_p) and _p not in sys.path:
        sys.path.append(_p)

import numpy as np

import concourse.tile as tile
from concourse import bass_utils, bacc, mybir

N, D, C = 32768, 512, 395  # batch, feat dim, classes
NCORES = 8
NSH = N // NCORES  # 4096 rows per core
KT = NSH // 128  # 32 K-tiles per core
GROUP_SIZES = [2, 4, 8, 8, 8, 1, 1]  # K-tiles per DMA group: small first so
# the matmuls start early, big middle for DMA bandwidth, small last so the
# final arrive-then-consume step is tiny
assert sum(GROUP_SIZES) == KT
OUT_ROWS = 8 * 128 + 1  # 4 D-chunks x 2 modals + counts row

_CACHE = {}


def _build(W):
    fp32 = mybir.dt.float32
    fp16 = mybir.dt.float16
    bf16 = mybir.dt.bfloat16
    nc = bacc.Bacc("TRN2", target_bir_lowering=False, debug=False, num_devices=NCORES)
    # x holds both modals row-interleaved: row r = [modal1[r, :] | modal2[r, :]]
    x = nc.dram_tensor("x", [NSH, 2 * D], bf16, kind="ExternalInput")
    tgt = nc.dram_tensor("tgt", [128, KT], fp32, kind="ExternalInput")
    iota = nc.dram_tensor("iota", [128, W], fp16, kind="ExternalInput")
    out = nc.dram_tensor("out", [OUT_ROWS, W], bf16, kind="ExternalOutput")

    with tile.TileContext(nc) as tc:
        with (
            tc.tile_pool(name="xin", bufs=6) as xin,
            tc.tile_pool(name="single", bufs=1) as single,
            tc.tile_pool(name="psum", bufs=1, space="PSUM") as psum,
        ):
            iota_sb = single.tile([128, W], fp16)
            nc.scalar.dma_start(iota_sb[:], iota.ap())
            tgt_sb = single.tile([128, KT], fp32)
            nc.scalar.dma_start(tgt_sb[:], tgt.ap())
            ones = single.tile([128, 1], bf16)
            nc.vector.memset(ones[:], 1.0)
            oh = single.tile([128, KT, W], bf16)  # all K-tile one-hots
            sums_sb = single.tile([128, 8, W], bf16)
            counts_sb = single.tile([1, W], bf16)

            # acc0's PSUM bank has spare columns (a bank is 512 f32); counts
            # accumulate in cols [W, 2W) of the same bank, row 0.
            accs = [
                psum.tile([128, 2 * W if j == 0 else W], fp32, tag=f"acc{j}", name=f"acc{j}")
                for j in range(8)
            ]

            # partition p holds rows p*KT..p*KT+KT-1 of the shard, so each
            # group DMA is a fully contiguous tpg*2KB run per partition
            xf = x.ap().rearrange("(p r) d -> p r d", p=128)

            k0 = 0
            for g, tpg in enumerate(GROUP_SIZES):
                xg = xin.tile([128, 8, 2 * D], bf16, tag="xg", name="xg")
                nc.sync.dma_start(xg[:, :tpg, :], xf[:, k0 : k0 + tpg, :])
                for t in range(tpg):
                    k = k0 + t
                    ohk = oh[:, k, :]
                    nc.vector.tensor_scalar(
                        ohk,
                        iota_sb[:],
                        tgt_sb[:, k : k + 1],
                        None,
                        mybir.AluOpType.is_equal,
                    )
                    st, sp = (k == 0), (k == KT - 1)
                    for j in range(8):
                        nc.tensor.matmul(
                            accs[j][:, :W] if j == 0 else accs[j][:],
                            lhsT=xg[:, t, j * 128 : (j + 1) * 128],
                            rhs=ohk,
                            start=st,
                            stop=sp,
                        )
                    # counts: ones.T @ onehot into acc0's spare columns.
                    # start=False always: the k==0 modal matmul already cleared
                    # this bank's has_written bits, so the first write lands as
                    # an overwrite and later ones accumulate.
                    nc.tensor.matmul(
                        accs[0][0:1, W : 2 * W],
                        lhsT=ones[:],
                        rhs=ohk,
                        start=False,
                        stop=sp,
                        skip_group_check=True,
                    )
                k0 += tpg

            out_ap = out.ap()
            for j in range(8):
                src_ap = accs[j][:, :W] if j == 0 else accs[j][:]
                if j % 2 == 0:
                    nc.vector.tensor_copy(sums_sb[:, j, :], src_ap)
                else:
                    nc.scalar.copy(sums_sb[:, j, :], src_ap)
            nc.vector.tensor_copy(counts_sb[:], accs[0][0:1, W : 2 * W])
            # one DMA for all 8 sum tiles, partition-major rows (p*8+j) so each
            # partition writes one contiguous 8*W run
            nc.sync.dma_start(
                out_ap[0:1024, :].rearrange("(p j) w -> p j w", p=128), sums_sb[:]
            )
            nc.scalar.dma_start(out_ap[1024:1025, :], counts_sb[:])

    nc.compile()
    return nc


def _get_nc(W):
    key = ("nc", W)
    if key not in _CACHE:
        _CACHE[key] = _build(W)
    return _CACHE[key]


def _make_in_maps(modal1, modal2, targets):
    import ml_dtypes

    tg = np.asarray(targets).astype(np.int64).reshape(N)
    perm = np.argsort(tg, kind="stable")
    tgs = tg[perm]
    xcat = np.empty((N, 2 * D), dtype=ml_dtypes.bfloat16)
    xcat[:, :D] = np.asarray(modal1).astype(ml_dtypes.bfloat16)[perm]
    xcat[:, D:] = np.asarray(modal2).astype(ml_dtypes.bfloat16)[perm]

    bases = [int(tgs[c * NSH]) for c in range(NCORES)]
    maxw = max(int(tgs[(c + 1) * NSH - 1]) - bases[c] + 1 for c in range(NCORES))
    W = max(64, ((maxw + 15) // 16) * 16)

    iota = np.ascontiguousarray(
        np.broadcast_to(np.arange(W, dtype=np.float16), (128, W))
    )
    in_maps = []
    for c in range(NCORES):
        rows = slice(c * NSH, (c + 1) * NSH)
        tgt_c = np.ascontiguousarray(
            (tgs[rows] - bases[c]).reshape(128, KT).astype(np.float32)
        )  # [128, KT]: partition p row t <-> shard row p*KT + t
        in_maps.append(
            {
                "x": np.ascontiguousarray(xcat[rows]),
                "tgt": tgt_c,
                "iota": iota,
            }
        )
    return in_maps, bases, W


def _epilogue(acc, centers):
    # acc: [1025, C+pad] float64 global sums; rows 0..511 modal1 (D-major),
    # rows 512..1023 modal2, row 1024 counts.
    counts = acc[1024, :C]
    clamp = np.maximum(counts, 1.0)
    cT = np.asarray(centers, dtype=np.float64).T  # [D, C]

    def sl1(x):
        d = np.abs(x)
        return np.where(d < 1.0, 0.5 * d * d, d - 0.5)

    total = 0.0
    for base in (0, 512):
        meanT = acc[base : base + 512, :C] / clamp
        total += (sl1(meanT - cT) * counts).sum()
    return np.float32(total / (N * D))


def _run(inputs, trace=False, tmpdir=None):
    in_maps, bases, W = _make_in_maps(
        inputs["modal1_inputs"], inputs["modal2_inputs"], inputs["targets"]
    )
    nc = _get_nc(W)
    kw = {}
    if trace:
        kw = {"trace": True, "tmpdir": tmpdir}
    res = bass_utils.run_bass_kernel_spmd(
        nc, in_maps, core_ids=list(range(NCORES)), **kw
    )
    acc = np.zeros((OUT_ROWS, C + W), dtype=np.float64)
    for c in range(NCORES):
        o = np.asarray(res.results[c]["out"], dtype=np.float64)
        sums = o[0:1024].reshape(128, 8, W).transpose(1, 0, 2).reshape(1024, W)
        acc[0:1024, bases[c] : bases[c] + W] += sums
        acc[1024, bases[c] : bases[c] + W] += o[1024]
    loss = _epilogue(acc, inputs["centers"])
    return loss, res


def kernel(**inputs) -> np.ndarray:
    loss, _ = _run(inputs)
    return loss


def kernel_profiled(**inputs):
    """Like kernel() but returns (loss, BassKernelResults) with NTFF trace."""
    import tempfile
    import types

    # antenv.axon_hooks is missing in this image; shim it so bass_utils can
    # find the NTFF profile hook, and keep artifacts local.
    if "antenv.axon_hooks" not in sys.modules:
        import antenv

        hooks_mod = types.ModuleType("antenv.axon_hooks")
        _h = [None]
        hooks_mod.set_axon_ntff_profile_hook = lambda h: _h.__setitem__(0, h)
        hooks_mod.get_axon_ntff_profile_hook = lambda: _h[0]
        sys.modules["antenv.axon_hooks"] = hooks_mod
        antenv.axon_hooks = hooks_mod
        try:
            from trn_agent_boot.trn_boot import _ntff_profile_via_ctypes

            hooks_mod.set_axon_ntff_profile_hook(
                _ntff_profile_via_ctypes("/opt/axon/libaxon_pjrt.so")
            )
        except Exception as e:
            print(f"profile hook setup failed: {e}", file=sys.stderr)
    bass_utils.upload_artifacts = lambda d: d
    tmpdir = tempfile.mkdtemp(prefix="ccloss_trace_")
    return _run(inputs, trace=True, tmpdir=tmpdir)
